# revision 12
# baseline (speedup 1.0000x reference)
"""6-layer GCN (GCNConv + ReLU) on 8 Trainium2 NeuronCores.

Sharding: nodes are partitioned across the 8 cores (6250 each, padded to
6272 = 49 blocks of 128). Weights are replicated. Each layer:
  dense:     xw = x_local @ W           (PE, fp32)
  allgather: xw_full = AG(xw)           (fp16 table, 12.8 MB)
  aggregate: x_next = relu(A_hat @ xw_full + b)
The aggregate is edge-parallel: edges sorted by (dst block, src half),
dma_gather pulls 128-edge chunks of source rows (fp16, 256 B rows), a
one-hot*norm selector S^T is built on-chip (iota + tensor_scalar), and
chunks are segment-summed into PSUM via matmul accumulation.
Layer 6 is computed as (A @ x6) @ W6 + b6 (math-equivalent reorder) so the
propagation stays 128-wide instead of 500-wide.
"""

import sys
import types

for _p in ("/opt/trn_rl_repo",):
    if _p not in sys.path:
        sys.path.insert(0, _p)

import numpy as np

N = 50000
E = 800000
F_IN = 500
H = 128
NCORES = 8
PER = N // NCORES            # 6250 real nodes per core
NB = 49                      # dst blocks of 128 per core
NL = NB * 128                # 6272 padded local nodes
NPAD = NCORES * NL           # 50176 padded global rows
HALF = NPAD // 2             # 25088 (< 32768 so indices fit int16)
KT = 4                       # K tiles for layer-1 dense (500 -> 512)


def _preprocess(x, edge_index):
    """Build per-core gather/selector tables and the shared chunk schedule."""
    src = np.asarray(edge_index[0], dtype=np.int64)
    dst = np.asarray(edge_index[1], dtype=np.int64)
    loops = np.arange(N, dtype=np.int64)
    src_all = np.concatenate([src, loops])
    dst_all = np.concatenate([dst, loops])

    deg = np.bincount(dst_all, minlength=N).astype(np.float64)
    dis = np.where(deg > 0, 1.0 / np.sqrt(deg), 0.0)
    norm = (dis[src_all] * dis[dst_all]).astype(np.float32)

    g_src = (src_all // PER) * NL + (src_all % PER)   # padded global row
    half = (g_src >= HALF).astype(np.int64)
    idx_in_half = (g_src - half * HALF).astype(np.int64)

    core = dst_all // PER
    dloc = dst_all % PER
    blk = dloc // 128
    drow = dloc % 128

    # group edges by (core, block, half) and count
    counts = np.zeros((NCORES, NB, 2), dtype=np.int64)
    per_core = []
    for r in range(NCORES):
        m = core == r
        b_r = blk[m]
        h_r = half[m]
        order = np.lexsort((h_r, b_r))
        per_core.append((idx_in_half[m][order], drow[m][order].astype(np.float32),
                         norm[m][order], b_r[order], h_r[order]))
        for b in range(NB):
            mb = b_r == b
            counts[r, b, 0] = np.sum(h_r[mb] == 0)
            counts[r, b, 1] = np.sum(h_r[mb] == 1)

    # shared chunk schedule: C[b][h] = chunks of 128 edges (max over cores)
    cmax = counts.max(axis=0)                      # [NB, 2]
    C = np.maximum((cmax + 127) // 128, 1).astype(np.int64)
    chunk_off = np.zeros((NB, 2), dtype=np.int64)
    t = 0
    for b in range(NB):
        for h in range(2):
            chunk_off[b, h] = t
            t += C[b, h]
    T = t                                          # total chunks per core

    gidx = np.zeros((NCORES, 128, T * 8), dtype=np.int16)
    dstrow = np.zeros((NCORES, 128, T), dtype=np.float32)
    normv = np.zeros((NCORES, 128, T), dtype=np.float32)

    for r in range(NCORES):
        idxs_r, drow_r, norm_r, b_r, h_r = per_core[r]
        pos = 0
        for b in range(NB):
            for h in range(2):
                n_e = counts[r, b, h]
                cap = C[b, h] * 128
                ge_idx = np.zeros(cap, dtype=np.int64)
                ge_drow = np.zeros(cap, dtype=np.float32)
                ge_norm = np.zeros(cap, dtype=np.float32)
                ge_idx[:n_e] = idxs_r[pos:pos + n_e]
                ge_drow[:n_e] = drow_r[pos:pos + n_e]
                ge_norm[:n_e] = norm_r[pos:pos + n_e]
                pos += n_e
                off8 = chunk_off[b, h] * 8
                w = ge_idx.astype(np.int16).reshape(C[b, h] * 8, 16).T  # [16, C*8]
                gidx[r, :, off8:off8 + C[b, h] * 8] = np.tile(w, (8, 1))
                toff = chunk_off[b, h]
                dstrow[r, :, toff:toff + C[b, h]] = \
                    ge_drow.reshape(C[b, h], 128).T
                normv[r, :, toff:toff + C[b, h]] = \
                    ge_norm.reshape(C[b, h], 128).T

    sched = [(b, h, int(C[b, h]), int(chunk_off[b, h])) for b in range(NB)
             for h in range(2)]
    return gidx, dstrow, normv, sched, T


def _build_program(sched, T, stages=99):
    """stages: 1=dense1+AG, 2=+agg1, 3=+layers2-4, 4=+layer5, 99=full."""
    import concourse.bacc as bacc
    import concourse.mybir as mybir
    import concourse.tile as tile

    dt = mybir.dt
    Alu = mybir.AluOpType
    Act = mybir.ActivationFunctionType

    nc = bacc.Bacc("TRN2", target_bir_lowering=False, debug=False,
                   num_devices=NCORES)

    xT_d = nc.dram_tensor("xT", [KT, 128, NL], dt.float32, kind="ExternalInput")
    W1_d = nc.dram_tensor("W1p", [128, KT, H], dt.float32, kind="ExternalInput")
    Wm_d = nc.dram_tensor("Wm", [128, 4, H], dt.float32, kind="ExternalInput")
    W6_d = nc.dram_tensor("W6p", [128, F_IN], dt.float32, kind="ExternalInput")
    bT_d = nc.dram_tensor("bT14", [128, 4], dt.float32, kind="ExternalInput")
    b5_d = nc.dram_tensor("b5bc", [128, H], dt.float32, kind="ExternalInput")
    b6_d = nc.dram_tensor("b6bc", [128, F_IN], dt.float32, kind="ExternalInput")
    gi_d = nc.dram_tensor("gidx", [128, T * 8], dt.int16, kind="ExternalInput")
    dr_d = nc.dram_tensor("dstrow", [128, T], dt.float32, kind="ExternalInput")
    nv_d = nc.dram_tensor("normv", [128, T], dt.float32, kind="ExternalInput")
    out_d = nc.dram_tensor("out", [NL, F_IN], dt.float32, kind="ExternalOutput")

    rg = [list(range(NCORES))]

    with tile.TileContext(nc) as tc:
        with (
            tc.tile_pool(name="const", bufs=1) as cp,
            tc.tile_pool(name="sbuf", bufs=2) as sb,
            tc.tile_pool(name="psum", bufs=3, space="PSUM") as pp,
            tc.tile_pool(name="dram", bufs=2, space="DRAM") as dp,
        ):
            # ---- static tables -------------------------------------------
            gi_t = cp.tile([128, T * 8], dt.int16)
            dr_t = cp.tile([128, T], dt.float32)
            nv_t = cp.tile([128, T], dt.float32)
            nc.sync.dma_start(out=gi_t[:], in_=gi_d[:])
            nc.sync.dma_start(out=dr_t[:], in_=dr_d[:])
            nc.sync.dma_start(out=nv_t[:], in_=nv_d[:])
            iota_t = cp.tile([128, 128], dt.float16)
            nc.gpsimd.iota(iota_t[:], pattern=[[1, 128]], base=0,
                           channel_multiplier=0,
                           allow_small_or_imprecise_dtypes=True)
            W1_t = cp.tile([128, KT, H], dt.float32)
            Wm_t = cp.tile([128, 4, H], dt.float32)
            W6_t = cp.tile([128, F_IN], dt.float32)
            bT_t = cp.tile([128, 4], dt.float32)
            b5_t = cp.tile([128, H], dt.float32)
            b6_t = cp.tile([128, F_IN], dt.float32)
            nc.sync.dma_start(out=W1_t[:], in_=W1_d[:])
            nc.sync.dma_start(out=Wm_t[:], in_=Wm_d[:])
            nc.sync.dma_start(out=W6_t[:], in_=W6_d[:])
            nc.sync.dma_start(out=bT_t[:], in_=bT_d[:])
            nc.sync.dma_start(out=b5_t[:], in_=b5_d[:])
            nc.sync.dma_start(out=b6_t[:], in_=b6_d[:])

            def stage_to_ag(stage_t):
                """stage [128, NB, H] fp16 -> new AG pair, returns ag_out."""
                ag_in = dp.tile([NL, H], dt.float16, tag="agin")
                ag_out = dp.tile([NPAD, H], dt.float16, tag="agout")
                nc.sync.dma_start(
                    out=ag_in[:].rearrange("(c p) f -> p c f", p=128),
                    in_=stage_t[:, :, :])
                nc.gpsimd.collective_compute(
                    "AllGather", Alu.bypass, replica_groups=rg,
                    ins=[ag_in.opt()], outs=[ag_out.opt()])
                return ag_out

            # ---- layer-1 dense: xw1 = x @ W1 (K-tiled over 4) ------------
            stage_t = sb.tile([128, NB, H], dt.float16, tag="stage")
            acc_t = sb.tile([128, NB, H], dt.float32, tag="acc", bufs=1)
            for k in range(KT):
                xk_t = sb.tile([128, NL], dt.float32, tag="xk", bufs=2)
                nc.sync.dma_start(out=xk_t[:], in_=xT_d[k])
                for n in range(NB):
                    ps = pp.tile([128, H], dt.float32, tag="dps", space="PSUM")
                    nc.tensor.matmul(out=ps[:], lhsT=xk_t[:, n*128:(n+1)*128],
                                     rhs=W1_t[:, k, :], start=True, stop=True)
                    if k == 0:
                        nc.vector.tensor_copy(out=acc_t[:, n, :], in_=ps[:])
                    elif k < KT - 1:
                        nc.vector.tensor_tensor(out=acc_t[:, n, :],
                                                in0=acc_t[:, n, :], in1=ps[:],
                                                op=Alu.add)
                    else:
                        nc.vector.tensor_tensor(out=stage_t[:, n, :],
                                                in0=acc_t[:, n, :], in1=ps[:],
                                                op=Alu.add)
            ag_out = stage_to_ag(stage_t)

            # ---- layers ---------------------------------------------------
            # layer l in 1..4: aggregate (transposed, bias,relu) -> dense l+1
            # layer 5: aggregate (node-major, bias,relu) -> AG -> layer 6
            # layer 6: aggregate (transposed, raw) -> dense6 -> out
            def aggregate(ag_src, transposed):
                """Yield (b, psum_tile) per dst block, accumulated over edges."""
                for b in range(NB):
                    groups = [(h, c_bh, coff) for (bb, h, c_bh, coff) in sched
                              if bb == b and c_bh > 0]
                    total = sum(c for (_, c, _) in groups)
                    ps = pp.tile([128, H], dt.float32, tag="aps", space="PSUM")
                    mm = 0
                    for h, c_bh, coff in groups:
                        g_t = sb.tile([128, c_bh, H], dt.float16, tag="gat",
                                      bufs=2)
                        nc.gpsimd.dma_gather(
                            out_ap=g_t[:, :, :],
                            in_ap=ag_src[h * HALF:(h + 1) * HALF, :],
                            idxs_ap=gi_t[:, coff * 8:(coff + c_bh) * 8],
                            num_idxs=c_bh * 128, num_idxs_reg=c_bh * 128,
                            elem_size=H,
                            single_packet=(c_bh * 128 <= 1024))
                        for c in range(c_bh):
                            st = sb.tile([128, 128], dt.float16, tag="st",
                                         bufs=6)
                            nc.vector.tensor_scalar(
                                out=st[:], in0=iota_t[:],
                                scalar1=dr_t[:, coff + c:coff + c + 1],
                                scalar2=nv_t[:, coff + c:coff + c + 1],
                                op0=Alu.is_equal, op1=Alu.mult)
                            first, last = mm == 0, mm == total - 1
                            if transposed:
                                nc.tensor.matmul(out=ps[:], lhsT=g_t[:, c, :],
                                                 rhs=st[:], start=first,
                                                 stop=last)
                            else:
                                nc.tensor.matmul(out=ps[:], lhsT=st[:],
                                                 rhs=g_t[:, c, :], start=first,
                                                 stop=last)
                            mm += 1
                    yield b, ps

            if stages >= 2:
                last_agg_layer = {2: 1, 3: 4, 4: 4}.get(stages, 4)
                for layer in range(1, last_agg_layer + 1):
                    # aggregate layer -> x_{l+1}^T [128 f, NL] fp32
                    xT_t = sb.tile([128, NL], dt.float32, tag="xT", bufs=2)
                    for b, ps in aggregate(ag_out, transposed=True):
                        nc.scalar.activation(out=xT_t[:, b*128:(b+1)*128],
                                             in_=ps[:], func=Act.Relu,
                                             bias=bT_t[:, layer-1:layer],
                                             scale=1.0)
                    # dense l+1
                    stage_t = sb.tile([128, NB, H], dt.float16, tag="stage")
                    for n in range(NB):
                        ps = pp.tile([128, H], dt.float32, tag="dps",
                                     space="PSUM")
                        nc.tensor.matmul(out=ps[:],
                                         lhsT=xT_t[:, n*128:(n+1)*128],
                                         rhs=Wm_t[:, layer - 1, :], start=True,
                                         stop=True)
                        nc.vector.tensor_copy(out=stage_t[:, n, :], in_=ps[:])
                    ag_out = stage_to_ag(stage_t)

            if stages >= 4:
                # layer 5 aggregate: node-major x6 = relu(A xw5 + b5)
                stage_t = sb.tile([128, NB, H], dt.float16, tag="stage")
                for b, ps in aggregate(ag_out, transposed=False):
                    tmp = sb.tile([128, H], dt.float32, tag="x6t", bufs=4)
                    nc.vector.tensor_tensor(out=tmp[:], in0=ps[:], in1=b5_t[:],
                                            op=Alu.add)
                    nc.vector.tensor_scalar(out=stage_t[:, b, :], in0=tmp[:],
                                            scalar1=0.0, scalar2=None,
                                            op0=Alu.max)
                ag_out = stage_to_ag(stage_t)

            if stages >= 5:
                # layer 6 aggregate (transposed, raw) -> z^T
                zT_t = sb.tile([128, NL], dt.float32, tag="xT", bufs=2)
                for b, ps in aggregate(ag_out, transposed=True):
                    nc.vector.tensor_copy(out=zT_t[:, b*128:(b+1)*128],
                                          in_=ps[:])

                # dense 6: out = relu(z @ W6 + b6)
                for n in range(NB):
                    ps6 = pp.tile([128, F_IN], dt.float32, tag="d6",
                                  space="PSUM", bufs=2)
                    nc.tensor.matmul(out=ps6[:], lhsT=zT_t[:, n*128:(n+1)*128],
                                     rhs=W6_t[:], start=True, stop=True)
                    o_t = sb.tile([128, F_IN], dt.float32, tag="ot", bufs=3)
                    nc.vector.tensor_tensor(out=o_t[:], in0=ps6[:],
                                            in1=b6_t[:], op=Alu.add)
                    nc.vector.tensor_scalar(out=o_t[:], in0=o_t[:],
                                            scalar1=0.0, scalar2=None,
                                            op0=Alu.max)
                    nc.sync.dma_start(out=out_d[n*128:(n+1)*128, :],
                                      in_=o_t[:])

    nc.compile()
    return nc


def _run(inputs, trace=False, stages=99):
    from concourse.bass_utils import run_bass_kernel_spmd

    x = np.asarray(inputs["x"], dtype=np.float32)
    edge_index = np.asarray(inputs["edge_index"])
    gidx, dstrow, normv, sched, T = _preprocess(x, edge_index)

    W1 = np.asarray(inputs["W1"], dtype=np.float32)
    W1p = np.zeros((KT * 128, H), np.float32)
    W1p[:F_IN] = W1
    W1p = W1p.reshape(KT, 128, H).transpose(1, 0, 2).copy()
    Wm = np.stack([np.asarray(inputs[f"W{i}"], dtype=np.float32)
                   for i in range(2, 6)], axis=1).copy()
    W6p = np.asarray(inputs["W6"], dtype=np.float32)
    bT14 = np.stack([np.asarray(inputs[f"b{i}"], dtype=np.float32)
                     for i in range(1, 5)], axis=1)
    b5bc = np.tile(np.asarray(inputs["b5"], dtype=np.float32)[None, :],
                   (128, 1))
    b6bc = np.tile(np.asarray(inputs["b6"], dtype=np.float32)[None, :],
                   (128, 1))

    in_maps = []
    for r in range(NCORES):
        xT = np.zeros((KT * 128, NL), np.float32)
        xT[:F_IN, :PER] = x[r * PER:(r + 1) * PER].T
        in_maps.append({
            "xT": xT.reshape(KT, 128, NL),
            "W1p": W1p, "Wm": Wm, "W6p": W6p,
            "bT14": bT14, "b5bc": b5bc, "b6bc": b6bc,
            "gidx": gidx[r], "dstrow": dstrow[r], "normv": normv[r],
        })

    nc = _build_program(sched, T, stages=stages)
    res = run_bass_kernel_spmd(nc, in_maps, core_ids=list(range(NCORES)),
                               trace=trace)
    out = np.concatenate([res.results[r]["out"][:PER] for r in range(NCORES)],
                         axis=0)
    return out, res.exec_time_ns


def kernel(**inputs):
    out, _ = _run(inputs, trace=False)
    return out
